# revision 24
# baseline (speedup 1.0000x reference)
"""PointerGuidance Trainium2 kernel.

Data-parallel over batch: 32 samples -> 8 NeuronCores x 4 samples.
Each core processes its 4 samples end-to-end; outputs are concatenated on host.

Device dataflow per core (feat shard [4, 256, 4096] f32):
  1. Box math + margin (tiny DVE/ACT ops on 4 partitions), boxes_xyxy out.
  2. Rectangle masks for all 9 conv-taps x 4 samples built as [36, 4096]
     indicator tensor via compares against host-precomputed shifted grids.
  3. conv3x3 (1->16) + relu and conv1x1 (16->1) + sigmoid as PE matmuls with
     block-diagonal weights -> refined_mask [4, 4096].
  4. Per feat tile [128, 4096]: ACT accumulates sum(feat); PE broadcasts the
     sample's mask row across 128 partitions (K=1 matmul into PSUM); one fused
     DVE affine_mul_reduce computes q = (0.6*mask+1)*feat in place and
     accumulates sum(q). pointer_feat = (sum(q)-sum(feat)) / (0.6*max(area,1)).
  5. Per-sample MLPs (channel gate + lang fuse) on PE with N=1 matmuls.
  6. In-place scale q *= (1+0.5*ch_gate) (per-partition scalar; split between
     ACT and DVE), DMA out.
"""

import os
import sys

import numpy as np

for _p in ("/opt/trn_rl_repo",):
    if os.path.isdir(_p) and _p not in sys.path:
        sys.path.insert(0, _p)

import concourse.bass as bass
import concourse.bacc as bacc
import concourse.mybir as mybir
import concourse.tile as tile
from concourse.bass_utils import run_bass_kernel_spmd

N_CORES = 8
B, C, H, W = 32, 256, 64, 64
BPC = B // N_CORES  # samples per core
HW = H * W
NCH = 2  # channel halves of 128

dt = mybir.dt
F32 = dt.float32
F32R = dt.float32r
BF16 = dt.bfloat16
Alu = mybir.AluOpType
Act = mybir.ActivationFunctionType

# jnp.linspace(0, 1, 64) == arange(64, f32) * f32(1/63) (verified bit-exact)
XS = (np.arange(64, dtype=np.float32) * np.float32(1.0 / 63.0)).astype(np.float32)
# sentinels for out-of-range taps: strictly below 0 / above 1 so the
# (>= lo) / (<= hi) compares are false for any clipped bound
XS_EXT = np.concatenate([[np.float32(-1.0)], XS, [np.float32(2.0)]]).astype(np.float32)

LAST_RESULTS = None  # BassKernelResults of the most recent run (for profiling)


def _build_grids():
    """[36, 4096] grids of shifted x / y linspace values (row j*4+b, tap j=(ky,kx))."""
    xg = np.empty((36, HW), np.float32)
    yg = np.empty((36, HW), np.float32)
    for ky in range(3):
        for kx in range(3):
            j = ky * 3 + kx
            xrow = np.tile(XS_EXT[kx : kx + 64], 64)  # value xs[x + (kx-1)]
            yrow = np.repeat(XS_EXT[ky : ky + 64], 64)  # value ys[y + (ky-1)]
            for b in range(BPC):
                xg[j * BPC + b] = xrow
                yg[j * BPC + b] = yrow
    return xg, yg


_XG36, _YG36 = _build_grids()


def _build_nc():
    nc = bacc.Bacc()

    # ---- DRAM I/O -------------------------------------------------------
    feat_d = nc.dram_tensor("feat", [BPC, C, HW], F32, kind="ExternalInput")
    xywh_d = nc.dram_tensor("xywh", [BPC, 4], F32, kind="ExternalInput")
    # cblob [64, 880]: c1lhsT | c2lhsT | c1b | c2b | ones128 | rep436 | id4 |
    #                  sel4 | xgs[36,64] | ygs[36,64]
    cblob_d = nc.dram_tensor("cblob", [64, 880], F32, kind="ExternalInput")
    # wblob [128, 3072]: wcg1T(4) | wlf1T(4) | wcg2T(2) | wlf2T(2), each [128,256]
    wblob_d = nc.dram_tensor("wblob", [128, 3072], F32, kind="ExternalInput")
    # bblob [128, 8]: bcg1 | bcg2 | blf1 | blf2, each [128,2]
    bblob_d = nc.dram_tensor("bblob", [128, 8], F32, kind="ExternalInput")
    # lblob [128, 8]: langT rows 0:128 | rows 128:256
    lblob_d = nc.dram_tensor("lblob", [128, 8], F32, kind="ExternalInput")

    guided_d = nc.dram_tensor("guided", [BPC, C, HW], F32, kind="ExternalOutput")
    glang_d = nc.dram_tensor("glangT", [C, BPC], F32, kind="ExternalOutput")
    boxes_d = nc.dram_tensor("boxes", [BPC, 4], F32, kind="ExternalOutput")
    rmask_d = nc.dram_tensor("rmask", [BPC, HW], F32, kind="ExternalOutput")
    pf_d = nc.dram_tensor("pfT", [C, BPC], F32, kind="ExternalOutput")

    v = nc.vector
    s = nc.scalar
    pe = nc.tensor

    with tile.TileContext(nc) as tc:
        with (
            tc.tile_pool(name="const", bufs=1) as cp,
            tc.tile_pool(name="tiny", bufs=1) as tp,
            tc.tile_pool(name="featp", bufs=8) as fp,
            tc.tile_pool(name="mskp", bufs=1) as bp,
            tc.tile_pool(name="hidp", bufs=1) as hp,
            tc.tile_pool(name="rmrp", bufs=1) as rp,
            tc.tile_pool(name="psA", bufs=3, space="PSUM") as psA,
            tc.tile_pool(name="psB", bufs=2, space="PSUM") as psB,
        ):
            # ---- load constants: grids first (they gate the mask
            # pipeline), then packed blobs (few DMA triggers) -------------
            xw = cp.tile([BPC, 4], F32, name="xw")
            nc.sync.dma_start(xw[:], xywh_d[:])
            cb = cp.tile([64, 880], F32, name="cb")
            nc.sync.dma_start(cb[:], cblob_d[:])
            c1l = cb[0:36, 0:64]
            c2l = cb[:, 64:68]
            c1b = cb[:, 68:69]
            c2b = cb[0:BPC, 69:70]
            ones = cb[0:1, 70:198]
            rep = cb[0:4, 198:234]
            id4 = cb[0:4, 234:238]
            sel2 = cb[0:36, 240:752]
            xgs = cb[0:36, 752:816]
            ygs = cb[0:36, 816:880]
            wb_ = cp.tile([128, 3072], F32, name="wb_")
            nc.sync.dma_start(wb_[:], wblob_d[:])
            wcg1 = [wb_[:, (k) * 256 : (k + 1) * 256] for k in range(4)]
            wlf1 = [wb_[:, (4 + k) * 256 : (5 + k) * 256] for k in range(4)]
            wcg2 = [wb_[:, (8 + k) * 256 : (9 + k) * 256] for k in range(2)]
            wlf2 = [wb_[:, (10 + k) * 256 : (11 + k) * 256] for k in range(2)]
            bb = cp.tile([128, 8], F32, name="bb")
            nc.sync.dma_start(bb[:], bblob_d[:])
            bcg1 = bb[:, 0:2]
            bcg2 = bb[:, 2:4]
            blf1 = bb[:, 4:6]
            blf2 = bb[:, 6:8]
            lb = cp.tile([128, 8], F32, name="lb")
            nc.sync.dma_start(lb[:], lblob_d[:])
            lT = [lb[:, 0:4], lb[:, 4:8]]

            # ---- box math (rows = samples, [4,1] columns) --------------
            def tnew(name):
                return tp.tile([BPC, 1], F32, name=name)

            def clamp01(dst, src):
                v.tensor_scalar(dst[:], src[:], 0.0, 1.0, Alu.max, Alu.min)

            xc, yc, wv, hv = (xw[:, i : i + 1] for i in range(4))
            hw_ = tnew("hw_")
            hh_ = tnew("hh_")
            v.tensor_scalar(hw_[:], wv, 0.5, None, Alu.mult)
            v.tensor_scalar(hh_[:], hv, 0.5, None, Alu.mult)
            x1, x2, y1, y2 = tnew("x1"), tnew("x2"), tnew("y1"), tnew("y2")
            v.tensor_sub(x1[:], xc, hw_[:])
            v.tensor_add(x2[:], xc, hw_[:])
            v.tensor_sub(y1[:], yc, hh_[:])
            v.tensor_add(y2[:], yc, hh_[:])
            x1c, x2c, y1c, y2c = tnew("x1c"), tnew("x2c"), tnew("y1c"), tnew("y2c")
            clamp01(x1c, x1)
            clamp01(x2c, x2)
            clamp01(y1c, y1)
            clamp01(y2c, y2)
            xlo, xhi, ylo, yhi = tnew("xlo"), tnew("xhi"), tnew("ylo"), tnew("yhi")
            v.tensor_tensor(xlo[:], x1c[:], x2c[:], Alu.min)
            v.tensor_max(xhi[:], x1c[:], x2c[:])
            v.tensor_tensor(ylo[:], y1c[:], y2c[:], Alu.min)
            v.tensor_max(yhi[:], y1c[:], y2c[:])
            wb, hb = tnew("wb"), tnew("hb")
            v.tensor_sub(wb[:], xhi[:], xlo[:])
            v.tensor_scalar(wb[:], wb[:], 1e-06, None, Alu.max)
            v.tensor_sub(hb[:], yhi[:], ylo[:])
            v.tensor_scalar(hb[:], hb[:], 1e-06, None, Alu.max)
            cx, cy = tnew("cx"), tnew("cy")
            v.tensor_add(cx[:], xhi[:], xlo[:])
            v.tensor_scalar(cx[:], cx[:], 0.5, None, Alu.mult)
            v.tensor_add(cy[:], yhi[:], ylo[:])
            v.tensor_scalar(cy[:], cy[:], 0.5, None, Alu.mult)
            wbh, hbh = tnew("wbh"), tnew("hbh")
            v.tensor_scalar(wbh[:], wb[:], 0.5, None, Alu.mult)
            v.tensor_scalar(hbh[:], hb[:], 0.5, None, Alu.mult)
            boxes_sb = tp.tile([BPC, 4], F32, name="boxes_sb")
            bx1, by1, bx2, by2 = (boxes_sb[:, i : i + 1] for i in range(4))
            tmp = tnew("tmpbox")
            v.tensor_sub(tmp[:], cx[:], wbh[:])
            clamp01(bx1, tmp)
            v.tensor_sub(tmp[:], cy[:], hbh[:])
            clamp01(by1, tmp)
            v.tensor_add(tmp[:], cx[:], wbh[:])
            clamp01(bx2, tmp)
            v.tensor_add(tmp[:], cy[:], hbh[:])
            clamp01(by2, tmp)
            nc.sync.dma_start(boxes_d[:], boxes_sb[:])

            # margin = clip(0.2*sqrt(w^2+h^2), 0.02, 0.2), w/h from clamped box
            wm, hm = tnew("wm"), tnew("hm")
            v.tensor_sub(wm[:], bx2, bx1)
            v.tensor_scalar(wm[:], wm[:], 1e-4, None, Alu.max)
            v.tensor_sub(hm[:], by2, by1)
            v.tensor_scalar(hm[:], hm[:], 1e-4, None, Alu.max)
            d2 = tnew("d2")
            v.tensor_mul(wm[:], wm[:], wm[:])
            v.tensor_mul(hm[:], hm[:], hm[:])
            v.tensor_add(d2[:], wm[:], hm[:])
            sq = tnew("sq")
            s.sqrt(sq[:], d2[:])
            # two Newton iterations: s <- 0.5*(s + d2/s), to match IEEE sqrt
            rcp = tnew("rcp")
            qn = tnew("qn")
            for _ in range(2):
                v.reciprocal(rcp[:], sq[:])
                v.tensor_mul(qn[:], d2[:], rcp[:])
                v.tensor_add(sq[:], sq[:], qn[:])
                v.tensor_scalar(sq[:], sq[:], 0.5, None, Alu.mult)
            margin = tnew("margin")
            v.tensor_scalar(margin[:], sq[:], 0.2, None, Alu.mult)
            v.tensor_scalar(margin[:], margin[:], 0.02, 0.2, Alu.max, Alu.min)
            bnd4 = tp.tile([BPC, 4], F32, name="bnd4")
            v.tensor_sub(tmp[:], bx1, margin[:])
            clamp01(bnd4[:, 0:1], tmp)
            v.tensor_add(tmp[:], bx2, margin[:])
            clamp01(bnd4[:, 1:2], tmp)
            v.tensor_sub(tmp[:], by1, margin[:])
            clamp01(bnd4[:, 2:3], tmp)
            v.tensor_add(tmp[:], by2, margin[:])
            clamp01(bnd4[:, 3:4], tmp)

            # replicate bounds to 36 partitions: rep.T @ bnd4
            bnd_ps = psB.tile([36, 4], F32, name="bnd_ps", tag="ps1")
            pe.matmul(bnd_ps[:], rep, bnd4[:], start=True, stop=True)
            bnd36 = tp.tile([36, 4], F32, name="bnd36")
            v.tensor_copy(bnd36[:], bnd_ps[:])

            # rounded copies of conv lhsTs and sel (tiny one-time ops)
            c1lr = tp.tile([36, 64], F32R, name="c1lr")
            v.tensor_copy(c1lr[:], c1l)
            c2lr = tp.tile([64, BPC], F32R, name="c2lr")
            v.tensor_copy(c2lr[:], c2l)
            sel2r = tp.tile([36, 4 * 128], BF16, name="sel2r")
            v.tensor_copy(sel2r[:], sel2)

            # ---- shifted rectangle masks: compare on [36,64] mini-grids,
            # expand to [36,4096] with stride-0 APs in one multiply -------
            xgate = tp.tile([36, 64], F32, name="xgate")
            ygate = tp.tile([36, 64], F32, name="ygate")
            v.tensor_scalar(xgate[:], xgs, bnd36[:, 0:1], None, Alu.is_ge)
            v.scalar_tensor_tensor(xgate[:], xgs, bnd36[:, 1:2], xgate[:], Alu.is_le, Alu.mult)
            v.tensor_scalar(ygate[:], ygs, bnd36[:, 2:3], None, Alu.is_ge)
            v.scalar_tensor_tensor(ygate[:], ygs, bnd36[:, 3:4], ygate[:], Alu.is_le, Alu.mult)
            msk = bp.tile([36, HW], F32R, name="msk", tag="mskt")
            xg_exp = bass.AP(xgate.tensor, xgate[:].offset, [xgate[:].ap[0], [0, 64], [1, 64]])
            yg_exp = bass.AP(ygate.tensor, ygate[:].offset, [ygate[:].ap[0], [1, 64], [0, 64]])
            v.tensor_tensor(msk[:].rearrange("p (a b) -> p a b", a=64), xg_exp, yg_exp, Alu.mult)

            # ---- conv1 3x3 (K=36 matmul) + relu ------------------------
            hidden = hp.tile([64, HW], F32R, name="hidden", tag="hid")
            for k in range(8):
                sl = slice(k * 512, (k + 1) * 512)
                ps = psB.tile([64, 512], F32, name="convps", tag="ps1")
                pe.matmul(ps[:], c1lr[:], msk[:, sl], start=True, stop=True)
                v.tensor_scalar(hidden[:, sl], ps[:], c1b, 0.0, Alu.add, Alu.max)

            # ---- conv2 1x1 (K=64 matmul) + sigmoid ---------------------
            rm4 = bp.tile([BPC, HW], F32, name="rm4", tag="mskt")
            for k in range(8):
                sl = slice(k * 512, (k + 1) * 512)
                ps2 = psB.tile([BPC, 512], F32, name="conv2ps", tag="ps1")
                pe.matmul(ps2[:], c2lr[:], hidden[:, sl], start=True, stop=True)
                s.activation(rm4[:, sl], ps2[:], Act.Sigmoid, bias=c2b)
            nc.sync.dma_start(rmask_d[:], rm4[:])
            # bf16 hi/lo split of the mask: hi+lo accumulated in PSUM by the
            # K=8 broadcast matmul recovers ~f32 precision at bf16 speed
            rmhl = rp.tile([36, HW], BF16, name="rmhl")
            nc.gpsimd.memset(rmhl[:], 0.0)
            v.tensor_copy(rmhl[0:BPC, :], rm4[:])
            v.tensor_sub(rmhl[32 : 32 + BPC, :], rm4[:], rmhl[0:BPC, :])
            rm4f = rm4[:]

            # ---- mask area -> u = 1/(0.6*max(area,1)) broadcast --------
            area = tp.tile([BPC, 1], F32, name="area")
            v.tensor_reduce(area[:], rm4f, mybir.AxisListType.X, Alu.add)
            v.tensor_scalar(area[:], area[:], 1.0, None, Alu.max)
            v.tensor_scalar(area[:], area[:], 0.6, None, Alu.mult)
            u4 = tnew("u4")
            v.reciprocal(u4[:], area[:])
            u_ps = psB.tile([1, 4], F32, name="u_ps", tag="ps1")
            pe.matmul(u_ps[:], u4[:], id4, start=True, stop=True)
            u_row = tp.tile([1, 4], F32, name="u_row")
            v.tensor_copy(u_row[:], u_ps[:])
            ubc_ps = psB.tile([128, 4], F32, name="ubc_ps", tag="ps1")
            pe.matmul(ubc_ps[:], ones, u_row[:], start=True, stop=True)
            u_bc = tp.tile([128, 4], F32, name="u_bc")
            v.tensor_copy(u_bc[:], ubc_ps[:])

            # ---- feat in + sum(feat) via in-place identity copy --------
            ft = {}
            sf = {}
            for b in range(BPC):
                for h in range(NCH):
                    t = fp.tile([128, HW], F32, name="ftile")
                    nc.sync.dma_start(t[:], feat_d[b, h * 128 : (h + 1) * 128, :])
                    ft[b, h] = t
                    acc = tp.tile([128, 1], F32, name=f"sf{b}{h}")
                    s.activation(t[:], t[:], Act.Copy, accum_out=acc[:])
                    sf[b, h] = acc

            # ---- main loop: broadcast mask, fused q & sum(q), MLPs -----
            fus = [tp.tile([128, BPC], F32, name=f"fus{h}") for h in range(2)]
            s1t = [tp.tile([128, BPC], F32, name=f"s1t{h}") for h in range(2)]
            glT = [tp.tile([128, BPC], F32, name=f"glT{m}") for m in range(2)]

            def mlp_pair(b0, n):
                """Batched MLPs for samples [b0, b0+n) (rhs columns b0:b0+n)."""
                rhs_ch = [fus[0], fus[1], lT[0], lT[1]]
                csl = slice(b0, b0 + n)

                def mlp(w1, w2, b1, b2, tag):
                    h1sb = []
                    for m in range(2):
                        h1ps = psB.tile([128, n], F32, name=f"h1ps{tag}{b0}{m}", tag="ps1")
                        for kc in range(4):
                            pe.matmul(
                                h1ps[:],
                                w1[kc][:, m * 128 : (m + 1) * 128],
                                rhs_ch[kc][:, csl],
                                start=(kc == 0),
                                stop=(kc == 3),
                            )
                        hh = tp.tile([128, n], F32, name=f"h1sb{tag}{b0}{m}")
                        s.activation(hh[:], h1ps[:], Act.Relu, bias=b1[:, m : m + 1])
                        h1sb.append(hh)
                    out2 = []
                    for m in range(2):
                        h2ps = psB.tile([128, n], F32, name=f"h2ps{tag}{b0}{m}", tag="ps1")
                        for kc in range(2):
                            pe.matmul(
                                h2ps[:],
                                w2[kc][:, m * 128 : (m + 1) * 128],
                                h1sb[kc][:],
                                start=(kc == 0),
                                stop=(kc == 1),
                            )
                        out2.append(h2ps)
                    return out2

                cg2 = mlp(wcg1, wcg2, bcg1, bcg2, "cg")
                for m in range(2):
                    g = tp.tile([128, n], F32, name=f"g{b0}{m}")
                    s.activation(g[:], cg2[m][:], Act.Sigmoid, bias=bcg2[:, m : m + 1])
                    v.tensor_scalar(s1t[m][:, csl], g[:], 0.5, 1.0, Alu.mult, Alu.add)
                lf2 = mlp(wlf1, wlf2, blf1, blf2, "lf")
                for m in range(2):
                    tn = tp.tile([128, n], F32, name=f"tn{b0}{m}")
                    s.activation(tn[:], lf2[m][:], Act.Tanh, bias=blf2[:, m : m + 1])
                    v.scalar_tensor_tensor(
                        glT[m][:, csl], tn[:], 0.4, lT[m][:, csl], Alu.mult, Alu.add
                    )

            def finish_sample(b):
                """Channel-gate scale (in place) + store guided tiles of b."""
                for h in range(NCH):
                    if b < 2:
                        s.activation(
                            ft[b, h][:], ft[b, h][:], Act.Copy, scale=s1t[h][:, b : b + 1]
                        )
                    else:
                        v.tensor_scalar(
                            ft[b, h][:], ft[b, h][:], s1t[h][:, b : b + 1], None, Alu.mult
                        )
                    nc.sync.dma_start(
                        guided_d[b, h * 128 : (h + 1) * 128, :], ft[b, h][:]
                    )

            for b in range(BPC):
                accq = {h: tp.tile([128, 4], F32, name=f"accq{b}{h}") for h in range(NCH)}
                for ck in range(4):
                    csl = slice(ck * 1024, (ck + 1) * 1024)
                    bc = psA.tile([128, 1024], F32, name="bc", tag="bc")
                    for s2 in range(2):
                        lo = ck * 1024 + s2 * 512
                        pe.matmul(
                            bc[:, s2 * 512 : (s2 + 1) * 512],
                            sel2r[:, b * 128 : (b + 1) * 128],
                            rmhl[:, lo : lo + 512],
                            start=True,
                            stop=True,
                        )
                    for h in range(NCH):
                        v.affine_mul_reduce(
                            out=ft[b, h][:, csl],
                            accum_out=accq[h][:, ck : ck + 1],
                            in0=bc[:],
                            in1=ft[b, h][:, csl],
                            scale=0.6,
                            bias=1.0,
                        )
                # pointer_feat column b: (sum q - sum feat) * u
                for h in range(NCH):
                    sq_ = tp.tile([128, 1], F32, name=f"sq{b}{h}")
                    v.tensor_reduce(sq_[:], accq[h][:], mybir.AxisListType.X, Alu.add)
                    v.tensor_sub(sq_[:], sq_[:], sf[b, h][:])
                    v.tensor_mul(fus[h][:, b : b + 1], sq_[:], u_bc[:, b : b + 1])
                if b == 1:
                    mlp_pair(0, 2)
                    finish_sample(0)
                    finish_sample(1)
                elif b == 3:
                    mlp_pair(2, 2)
                    finish_sample(2)
                    finish_sample(3)

            for m in range(2):
                nc.sync.dma_start(glang_d[m * 128 : (m + 1) * 128, :], glT[m][:])
                nc.sync.dma_start(pf_d[m * 128 : (m + 1) * 128, :], fus[m][:])

    nc.finalize()
    return nc


_NC_CACHE = None


def _get_nc():
    global _NC_CACHE
    if _NC_CACHE is None:
        _NC_CACHE = _build_nc()
    return _NC_CACHE


def _prep_in_maps(feat, lang_vec, pointer_xywh, conv1_w, conv1_b, conv2_w, conv2_b,
                  cg_w1, cg_b1, cg_w2, cg_b2, lf_w1, lf_b1, lf_w2, lf_b2):
    f32 = np.float32
    feat = np.ascontiguousarray(np.asarray(feat, f32).reshape(B, C, HW))
    lang_vec = np.asarray(lang_vec, f32)
    pointer_xywh = np.asarray(pointer_xywh, f32)

    w1flat = np.asarray(conv1_w, f32).reshape(16, 9)
    w2vec = np.asarray(conv2_w, f32).reshape(16)

    cblob = np.zeros((64, 880), f32)
    for b in range(BPC):
        for j in range(9):
            cblob[j * BPC + b, b * 16 : (b + 1) * 16] = w1flat[:, j]   # c1lhsT
        cblob[b * 16 : (b + 1) * 16, 64 + b] = w2vec                   # c2lhsT
        cblob[b, 69] = np.asarray(conv2_b, f32)[0]                     # c2b
        for j in range(9):
            cblob[b, 198 + j * BPC + b] = 1.0                          # rep436
        cblob[b, 234 + b] = 1.0                                        # id4
        cblob[b, 240 + b * 128 : 240 + (b + 1) * 128] = 1.0            # sel2 hi
        cblob[32 + b, 240 + b * 128 : 240 + (b + 1) * 128] = 1.0       # sel2 lo
    cblob[:, 68] = np.tile(np.asarray(conv1_b, f32), BPC)              # c1b
    cblob[0, 70:198] = 1.0                                             # ones128
    for ky in range(3):
        for kx in range(3):
            j = ky * 3 + kx
            for b in range(BPC):
                cblob[j * BPC + b, 752:816] = XS_EXT[kx : kx + 64]     # xgs
                cblob[j * BPC + b, 816:880] = XS_EXT[ky : ky + 64]     # ygs

    wblob = np.empty((128, 3072), f32)
    mats = ([np.asarray(cg_w1, f32).T[k * 128 : (k + 1) * 128] for k in range(4)]
            + [np.asarray(lf_w1, f32).T[k * 128 : (k + 1) * 128] for k in range(4)]
            + [np.asarray(cg_w2, f32).T[k * 128 : (k + 1) * 128] for k in range(2)]
            + [np.asarray(lf_w2, f32).T[k * 128 : (k + 1) * 128] for k in range(2)])
    for k, m in enumerate(mats):
        wblob[:, k * 256 : (k + 1) * 256] = m

    bblob = np.empty((128, 8), f32)
    for i, bv in enumerate([cg_b1, cg_b2, lf_b1, lf_b2]):
        bblob[:, 2 * i : 2 * i + 2] = np.asarray(bv, f32).reshape(2, 128).T

    com = {
        "cblob": cblob,
        "wblob": wblob,
        "bblob": bblob,
    }
    in_maps = []
    for i in range(N_CORES):
        sl = slice(i * BPC, (i + 1) * BPC)
        m = dict(com)
        m["feat"] = np.ascontiguousarray(feat[sl])
        m["xywh"] = np.ascontiguousarray(pointer_xywh[sl])
        lT = np.ascontiguousarray(lang_vec[sl].T)  # [256, 4]
        lblob = np.empty((128, 8), f32)
        lblob[:, 0:4] = lT[0:128]
        lblob[:, 4:8] = lT[128:256]
        m["lblob"] = lblob
        in_maps.append(m)
    return in_maps


def _gather(results):
    f32 = np.float32
    guided = np.concatenate([r["guided"] for r in results], 0).reshape(B, C, H, W)
    glang = np.concatenate([r["glangT"].T for r in results], 0)
    boxes = np.concatenate([r["boxes"] for r in results], 0)
    rmask = np.concatenate([r["rmask"] for r in results], 0).reshape(B, 1, H, W)
    pf = np.concatenate([r["pfT"].T for r in results], 0)
    return (
        guided.astype(f32, copy=False),
        glang.astype(f32, copy=False),
        boxes.astype(f32, copy=False),
        rmask.astype(f32, copy=False),
        pf.astype(f32, copy=False),
    )


def kernel(**inputs):
    global LAST_RESULTS
    nc = _get_nc()
    in_maps = _prep_in_maps(**inputs)
    res = run_bass_kernel_spmd(nc, in_maps, core_ids=list(range(N_CORES)))
    LAST_RESULTS = res
    return _gather(res.results)


# revision 27
# speedup vs baseline: 1.0503x; 1.0503x over previous
"""PointerGuidance Trainium2 kernel.

Data-parallel over batch: 32 samples -> 8 NeuronCores x 4 samples.
Each core processes its 4 samples end-to-end; outputs are concatenated on host.

Device dataflow per core (feat shard [4, 256, 4096] f32):
  1. Box math + margin (tiny DVE/ACT ops on 4 partitions), boxes_xyxy out.
  2. Rectangle masks for all 9 conv-taps x 4 samples built as [36, 4096]
     indicator tensor via compares against host-precomputed shifted grids.
  3. conv3x3 (1->16) + relu and conv1x1 (16->1) + sigmoid as PE matmuls with
     block-diagonal weights -> refined_mask [4, 4096].
  4. Per feat tile [128, 4096]: ACT accumulates sum(feat); PE broadcasts the
     sample's mask row across 128 partitions (K=1 matmul into PSUM); one fused
     DVE affine_mul_reduce computes q = (0.6*mask+1)*feat in place and
     accumulates sum(q). pointer_feat = (sum(q)-sum(feat)) / (0.6*max(area,1)).
  5. Per-sample MLPs (channel gate + lang fuse) on PE with N=1 matmuls.
  6. In-place scale q *= (1+0.5*ch_gate) (per-partition scalar; split between
     ACT and DVE), DMA out.
"""

import os
import sys

import numpy as np
import ml_dtypes

for _p in ("/opt/trn_rl_repo",):
    if os.path.isdir(_p) and _p not in sys.path:
        sys.path.insert(0, _p)

import concourse.bass as bass
import concourse.bacc as bacc
import concourse.mybir as mybir
import concourse.tile as tile
from concourse.bass_utils import run_bass_kernel_spmd

N_CORES = 8
B, C, H, W = 32, 256, 64, 64
BPC = B // N_CORES  # samples per core
HW = H * W
NCH = 2  # channel halves of 128

dt = mybir.dt
F32 = dt.float32
F32R = dt.float32r
BF16 = dt.bfloat16
Alu = mybir.AluOpType
Act = mybir.ActivationFunctionType

# jnp.linspace(0, 1, 64) == arange(64, f32) * f32(1/63) (verified bit-exact)
XS = (np.arange(64, dtype=np.float32) * np.float32(1.0 / 63.0)).astype(np.float32)
# sentinels for out-of-range taps: strictly below 0 / above 1 so the
# (>= lo) / (<= hi) compares are false for any clipped bound
XS_EXT = np.concatenate([[np.float32(-1.0)], XS, [np.float32(2.0)]]).astype(np.float32)

LAST_RESULTS = None  # BassKernelResults of the most recent run (for profiling)


def _build_grids():
    """[36, 4096] grids of shifted x / y linspace values (row j*4+b, tap j=(ky,kx))."""
    xg = np.empty((36, HW), np.float32)
    yg = np.empty((36, HW), np.float32)
    for ky in range(3):
        for kx in range(3):
            j = ky * 3 + kx
            xrow = np.tile(XS_EXT[kx : kx + 64], 64)  # value xs[x + (kx-1)]
            yrow = np.repeat(XS_EXT[ky : ky + 64], 64)  # value ys[y + (ky-1)]
            for b in range(BPC):
                xg[j * BPC + b] = xrow
                yg[j * BPC + b] = yrow
    return xg, yg


_XG36, _YG36 = _build_grids()


def _build_nc():
    nc = bacc.Bacc()

    # ---- DRAM I/O -------------------------------------------------------
    feat_d = nc.dram_tensor("feat", [BPC, C, HW], F32, kind="ExternalInput")
    xywh_d = nc.dram_tensor("xywh", [BPC, 4], F32, kind="ExternalInput")
    # cblob [64, 880]: c1lhsT | c2lhsT | c1b | c2b | ones128 | rep436 | id4 |
    #                  sel4 | xgs[36,64] | ygs[36,64]
    cblob_d = nc.dram_tensor("cblob", [64, 880], F32, kind="ExternalInput")
    # wblob [128, 3072]: wcg1T(4) | wlf1T(4) | wcg2T(2) | wlf2T(2), each [128,256]
    wblob_d = nc.dram_tensor("wblob", [128, 3072], BF16, kind="ExternalInput")
    # bblob [128, 8]: bcg1 | bcg2 | blf1 | blf2, each [128,2]
    bblob_d = nc.dram_tensor("bblob", [128, 8], F32, kind="ExternalInput")
    # lblob [128, 8]: langT rows 0:128 | rows 128:256
    lblob_d = nc.dram_tensor("lblob", [128, 8], F32, kind="ExternalInput")

    guided_d = nc.dram_tensor("guided", [BPC, C, HW], F32, kind="ExternalOutput")
    glang_d = nc.dram_tensor("glangT", [C, BPC], F32, kind="ExternalOutput")
    boxes_d = nc.dram_tensor("boxes", [BPC, 4], F32, kind="ExternalOutput")
    rmask_d = nc.dram_tensor("rmask", [BPC, HW], F32, kind="ExternalOutput")
    pf_d = nc.dram_tensor("pfT", [C, BPC], F32, kind="ExternalOutput")

    v = nc.vector
    s = nc.scalar
    pe = nc.tensor

    with tile.TileContext(nc) as tc:
        with (
            tc.tile_pool(name="const", bufs=1) as cp,
            tc.tile_pool(name="tiny", bufs=1) as tp,
            tc.tile_pool(name="featp", bufs=8) as fp,
            tc.tile_pool(name="mskp", bufs=1) as bp,
            tc.tile_pool(name="hidp", bufs=1) as hp,
            tc.tile_pool(name="rmrp", bufs=1) as rp,
            tc.tile_pool(name="psA", bufs=3, space="PSUM") as psA,
            tc.tile_pool(name="psB", bufs=2, space="PSUM") as psB,
        ):
            # ---- load constants: grids first (they gate the mask
            # pipeline), then packed blobs (few DMA triggers) -------------
            xw = cp.tile([BPC, 4], F32, name="xw")
            nc.sync.dma_start(xw[:], xywh_d[:])
            cb = cp.tile([64, 880], F32, name="cb")
            nc.sync.dma_start(cb[:], cblob_d[:])
            c1l = cb[0:36, 0:64]
            c2l = cb[:, 64:68]
            c1b = cb[:, 68:69]
            c2b = cb[0:BPC, 69:70]
            ones = cb[0:1, 70:198]
            rep = cb[0:4, 198:234]
            id4 = cb[0:4, 234:238]
            sel2 = cb[0:36, 240:752]
            xgs = cb[0:36, 752:816]
            ygs = cb[0:36, 816:880]
            wb_ = cp.tile([128, 3072], BF16, name="wb_")
            nc.sync.dma_start(wb_[:], wblob_d[:])
            wcg1 = [wb_[:, (k) * 256 : (k + 1) * 256] for k in range(4)]
            wlf1 = [wb_[:, (4 + k) * 256 : (5 + k) * 256] for k in range(4)]
            wcg2 = [wb_[:, (8 + k) * 256 : (9 + k) * 256] for k in range(2)]
            wlf2 = [wb_[:, (10 + k) * 256 : (11 + k) * 256] for k in range(2)]
            bb = cp.tile([128, 8], F32, name="bb")
            nc.sync.dma_start(bb[:], bblob_d[:])
            bcg1 = bb[:, 0:2]
            bcg2 = bb[:, 2:4]
            blf1 = bb[:, 4:6]
            blf2 = bb[:, 6:8]
            lb = cp.tile([128, 8], F32, name="lb")
            nc.sync.dma_start(lb[:], lblob_d[:])
            lT = [lb[:, 0:4], lb[:, 4:8]]
            lbb = tp.tile([128, 8], BF16, name="lbb")
            v.tensor_copy(lbb[:], lb[:])
            lTb = [lbb[:, 0:4], lbb[:, 4:8]]

            # ---- box math (rows = samples, [4,1] columns) --------------
            def tnew(name):
                return tp.tile([BPC, 1], F32, name=name)

            def clamp01(dst, src):
                v.tensor_scalar(dst[:], src[:], 0.0, 1.0, Alu.max, Alu.min)

            xc, yc, wv, hv = (xw[:, i : i + 1] for i in range(4))
            hw_ = tnew("hw_")
            hh_ = tnew("hh_")
            v.tensor_scalar(hw_[:], wv, 0.5, None, Alu.mult)
            v.tensor_scalar(hh_[:], hv, 0.5, None, Alu.mult)
            x1, x2, y1, y2 = tnew("x1"), tnew("x2"), tnew("y1"), tnew("y2")
            v.tensor_sub(x1[:], xc, hw_[:])
            v.tensor_add(x2[:], xc, hw_[:])
            v.tensor_sub(y1[:], yc, hh_[:])
            v.tensor_add(y2[:], yc, hh_[:])
            x1c, x2c, y1c, y2c = tnew("x1c"), tnew("x2c"), tnew("y1c"), tnew("y2c")
            clamp01(x1c, x1)
            clamp01(x2c, x2)
            clamp01(y1c, y1)
            clamp01(y2c, y2)
            xlo, xhi, ylo, yhi = tnew("xlo"), tnew("xhi"), tnew("ylo"), tnew("yhi")
            v.tensor_tensor(xlo[:], x1c[:], x2c[:], Alu.min)
            v.tensor_max(xhi[:], x1c[:], x2c[:])
            v.tensor_tensor(ylo[:], y1c[:], y2c[:], Alu.min)
            v.tensor_max(yhi[:], y1c[:], y2c[:])
            wb, hb = tnew("wb"), tnew("hb")
            v.tensor_sub(wb[:], xhi[:], xlo[:])
            v.tensor_scalar(wb[:], wb[:], 1e-06, None, Alu.max)
            v.tensor_sub(hb[:], yhi[:], ylo[:])
            v.tensor_scalar(hb[:], hb[:], 1e-06, None, Alu.max)
            cx, cy = tnew("cx"), tnew("cy")
            v.tensor_add(cx[:], xhi[:], xlo[:])
            v.tensor_scalar(cx[:], cx[:], 0.5, None, Alu.mult)
            v.tensor_add(cy[:], yhi[:], ylo[:])
            v.tensor_scalar(cy[:], cy[:], 0.5, None, Alu.mult)
            wbh, hbh = tnew("wbh"), tnew("hbh")
            v.tensor_scalar(wbh[:], wb[:], 0.5, None, Alu.mult)
            v.tensor_scalar(hbh[:], hb[:], 0.5, None, Alu.mult)
            boxes_sb = tp.tile([BPC, 4], F32, name="boxes_sb")
            bx1, by1, bx2, by2 = (boxes_sb[:, i : i + 1] for i in range(4))
            tmp = tnew("tmpbox")
            v.tensor_sub(tmp[:], cx[:], wbh[:])
            clamp01(bx1, tmp)
            v.tensor_sub(tmp[:], cy[:], hbh[:])
            clamp01(by1, tmp)
            v.tensor_add(tmp[:], cx[:], wbh[:])
            clamp01(bx2, tmp)
            v.tensor_add(tmp[:], cy[:], hbh[:])
            clamp01(by2, tmp)
            nc.sync.dma_start(boxes_d[:], boxes_sb[:])

            # margin = clip(0.2*sqrt(w^2+h^2), 0.02, 0.2), w/h from clamped box
            wm, hm = tnew("wm"), tnew("hm")
            v.tensor_sub(wm[:], bx2, bx1)
            v.tensor_scalar(wm[:], wm[:], 1e-4, None, Alu.max)
            v.tensor_sub(hm[:], by2, by1)
            v.tensor_scalar(hm[:], hm[:], 1e-4, None, Alu.max)
            d2 = tnew("d2")
            v.tensor_mul(wm[:], wm[:], wm[:])
            v.tensor_mul(hm[:], hm[:], hm[:])
            v.tensor_add(d2[:], wm[:], hm[:])
            sq = tnew("sq")
            s.sqrt(sq[:], d2[:])
            # two Newton iterations: s <- 0.5*(s + d2/s), to match IEEE sqrt
            rcp = tnew("rcp")
            qn = tnew("qn")
            for _ in range(2):
                v.reciprocal(rcp[:], sq[:])
                v.tensor_mul(qn[:], d2[:], rcp[:])
                v.tensor_add(sq[:], sq[:], qn[:])
                v.tensor_scalar(sq[:], sq[:], 0.5, None, Alu.mult)
            margin = tnew("margin")
            v.tensor_scalar(margin[:], sq[:], 0.2, None, Alu.mult)
            v.tensor_scalar(margin[:], margin[:], 0.02, 0.2, Alu.max, Alu.min)
            bnd4 = tp.tile([BPC, 4], F32, name="bnd4")
            v.tensor_sub(tmp[:], bx1, margin[:])
            clamp01(bnd4[:, 0:1], tmp)
            v.tensor_add(tmp[:], bx2, margin[:])
            clamp01(bnd4[:, 1:2], tmp)
            v.tensor_sub(tmp[:], by1, margin[:])
            clamp01(bnd4[:, 2:3], tmp)
            v.tensor_add(tmp[:], by2, margin[:])
            clamp01(bnd4[:, 3:4], tmp)

            # replicate bounds to 36 partitions: rep.T @ bnd4
            bnd_ps = psB.tile([36, 4], F32, name="bnd_ps", tag="ps1")
            pe.matmul(bnd_ps[:], rep, bnd4[:], start=True, stop=True)
            bnd36 = tp.tile([36, 4], F32, name="bnd36")
            v.tensor_copy(bnd36[:], bnd_ps[:])

            # rounded copies of conv lhsTs and sel (tiny one-time ops)
            c1lr = tp.tile([36, 64], F32R, name="c1lr")
            v.tensor_copy(c1lr[:], c1l)
            c2lr = tp.tile([64, BPC], F32R, name="c2lr")
            v.tensor_copy(c2lr[:], c2l)
            sel2r = tp.tile([36, 4 * 128], BF16, name="sel2r")
            v.tensor_copy(sel2r[:], sel2)

            # ---- shifted rectangle masks: compare on [36,64] mini-grids,
            # expand to [36,4096] with stride-0 APs in one multiply -------
            xgate = tp.tile([36, 64], F32, name="xgate")
            ygate = tp.tile([36, 64], F32, name="ygate")
            v.tensor_scalar(xgate[:], xgs, bnd36[:, 0:1], None, Alu.is_ge)
            v.scalar_tensor_tensor(xgate[:], xgs, bnd36[:, 1:2], xgate[:], Alu.is_le, Alu.mult)
            v.tensor_scalar(ygate[:], ygs, bnd36[:, 2:3], None, Alu.is_ge)
            v.scalar_tensor_tensor(ygate[:], ygs, bnd36[:, 3:4], ygate[:], Alu.is_le, Alu.mult)
            msk = bp.tile([36, HW], F32R, name="msk", tag="mskt")
            xg_exp = bass.AP(xgate.tensor, xgate[:].offset, [xgate[:].ap[0], [0, 64], [1, 64]])
            yg_exp = bass.AP(ygate.tensor, ygate[:].offset, [ygate[:].ap[0], [1, 64], [0, 64]])
            v.tensor_tensor(msk[:].rearrange("p (a b) -> p a b", a=64), xg_exp, yg_exp, Alu.mult)

            # ---- conv1 3x3 (K=36 matmul) + relu ------------------------
            hidden = hp.tile([64, HW], F32R, name="hidden", tag="hid")
            for k in range(8):
                sl = slice(k * 512, (k + 1) * 512)
                ps = psB.tile([64, 512], F32, name="convps", tag="ps1")
                pe.matmul(ps[:], c1lr[:], msk[:, sl], start=True, stop=True)
                v.tensor_scalar(hidden[:, sl], ps[:], c1b, 0.0, Alu.add, Alu.max)

            # ---- conv2 1x1 (K=64 matmul) + sigmoid ---------------------
            rm4 = bp.tile([BPC, HW], F32, name="rm4", tag="mskt")
            for k in range(8):
                sl = slice(k * 512, (k + 1) * 512)
                ps2 = psB.tile([BPC, 512], F32, name="conv2ps", tag="ps1")
                pe.matmul(ps2[:], c2lr[:], hidden[:, sl], start=True, stop=True)
                s.activation(rm4[:, sl], ps2[:], Act.Sigmoid, bias=c2b)
            nc.sync.dma_start(rmask_d[:], rm4[:])
            # bf16 hi/lo split of the mask: hi+lo accumulated in PSUM by the
            # K=8 broadcast matmul recovers ~f32 precision at bf16 speed
            rmhl = rp.tile([36, HW], BF16, name="rmhl")
            nc.gpsimd.memset(rmhl[:], 0.0)
            area = tp.tile([BPC, 1], F32, name="area")
            s.activation(rmhl[0:BPC, :], rm4[:], Act.Copy, accum_out=area[:])
            v.tensor_sub(rmhl[32 : 32 + BPC, :], rm4[:], rmhl[0:BPC, :])

            # ---- mask area -> u = 1/(0.6*max(area,1)) broadcast --------
            v.tensor_scalar(area[:], area[:], 1.0, None, Alu.max)
            v.tensor_scalar(area[:], area[:], 0.6, None, Alu.mult)
            u4 = tnew("u4")
            v.reciprocal(u4[:], area[:])
            u_ps = psB.tile([1, 4], F32, name="u_ps", tag="ps1")
            pe.matmul(u_ps[:], u4[:], id4, start=True, stop=True)
            u_row = tp.tile([1, 4], F32, name="u_row")
            v.tensor_copy(u_row[:], u_ps[:])
            ubc_ps = psB.tile([128, 4], F32, name="ubc_ps", tag="ps1")
            pe.matmul(ubc_ps[:], ones, u_row[:], start=True, stop=True)
            u_bc = tp.tile([128, 4], F32, name="u_bc")
            v.tensor_copy(u_bc[:], ubc_ps[:])

            # ---- feat in + sum(feat) via in-place identity copy --------
            ft = {}
            sf = {}
            for b in range(BPC):
                for h in range(NCH):
                    t = fp.tile([128, HW], F32, name="ftile")
                    nc.sync.dma_start(t[:], feat_d[b, h * 128 : (h + 1) * 128, :])
                    ft[b, h] = t
                    acc = tp.tile([128, 1], F32, name=f"sf{b}{h}")
                    s.activation(t[:], t[:], Act.Copy, accum_out=acc[:])
                    sf[b, h] = acc

            # ---- main loop: broadcast mask, fused q & sum(q), MLPs -----
            fus = [tp.tile([128, BPC], BF16, name=f"fus{h}") for h in range(2)]
            pfT = [tp.tile([128, BPC], F32, name=f"pfT{h}") for h in range(2)]
            s1t = [tp.tile([128, BPC], F32, name=f"s1t{h}") for h in range(2)]
            glT = [tp.tile([128, BPC], F32, name=f"glT{m}") for m in range(2)]

            def mlp_pair(b0, n):
                """Batched MLPs for samples [b0, b0+n) (rhs columns b0:b0+n)."""
                rhs_ch = [fus[0], fus[1], lTb[0], lTb[1]]
                csl = slice(b0, b0 + n)

                def mlp(w1, w2, b1, b2, tag):
                    h1sb = []
                    for m in range(2):
                        h1ps = psB.tile([128, n], F32, name=f"h1ps{tag}{b0}{m}", tag="ps1")
                        for kc in range(4):
                            pe.matmul(
                                h1ps[:],
                                w1[kc][:, m * 128 : (m + 1) * 128],
                                rhs_ch[kc][:, csl],
                                start=(kc == 0),
                                stop=(kc == 3),
                            )
                        hh = tp.tile([128, n], BF16, name=f"h1sb{tag}{b0}{m}")
                        s.activation(hh[:], h1ps[:], Act.Relu, bias=b1[:, m : m + 1])
                        h1sb.append(hh)
                    out2 = []
                    for m in range(2):
                        h2ps = psB.tile([128, n], F32, name=f"h2ps{tag}{b0}{m}", tag="ps1")
                        for kc in range(2):
                            pe.matmul(
                                h2ps[:],
                                w2[kc][:, m * 128 : (m + 1) * 128],
                                h1sb[kc][:],
                                start=(kc == 0),
                                stop=(kc == 1),
                            )
                        out2.append(h2ps)
                    return out2

                cg2 = mlp(wcg1, wcg2, bcg1, bcg2, "cg")
                for m in range(2):
                    g = tp.tile([128, n], F32, name=f"g{b0}{m}")
                    s.activation(g[:], cg2[m][:], Act.Sigmoid, bias=bcg2[:, m : m + 1])
                    v.tensor_scalar(s1t[m][:, csl], g[:], 0.5, 1.0, Alu.mult, Alu.add)
                lf2 = mlp(wlf1, wlf2, blf1, blf2, "lf")
                for m in range(2):
                    tn = tp.tile([128, n], F32, name=f"tn{b0}{m}")
                    s.activation(tn[:], lf2[m][:], Act.Tanh, bias=blf2[:, m : m + 1])
                    v.scalar_tensor_tensor(
                        glT[m][:, csl], tn[:], 0.4, lT[m][:, csl], Alu.mult, Alu.add
                    )

            def finish_sample(b):
                """Channel-gate scale (in place) + store guided tiles of b."""
                for h in range(NCH):
                    if b < 2:
                        s.activation(
                            ft[b, h][:], ft[b, h][:], Act.Copy, scale=s1t[h][:, b : b + 1]
                        )
                    else:
                        v.tensor_scalar(
                            ft[b, h][:], ft[b, h][:], s1t[h][:, b : b + 1], None, Alu.mult
                        )
                    nc.sync.dma_start(
                        guided_d[b, h * 128 : (h + 1) * 128, :], ft[b, h][:]
                    )

            for b in range(BPC):
                accq = {h: tp.tile([128, 4], F32, name=f"accq{b}{h}") for h in range(NCH)}
                for ck in range(4):
                    csl = slice(ck * 1024, (ck + 1) * 1024)
                    bc = psA.tile([128, 1024], F32, name="bc", tag="bc")
                    for s2 in range(2):
                        lo = ck * 1024 + s2 * 512
                        pe.matmul(
                            bc[:, s2 * 512 : (s2 + 1) * 512],
                            sel2r[:, b * 128 : (b + 1) * 128],
                            rmhl[:, lo : lo + 512],
                            start=True,
                            stop=True,
                        )
                    for h in range(NCH):
                        v.affine_mul_reduce(
                            out=ft[b, h][:, csl],
                            accum_out=accq[h][:, ck : ck + 1],
                            in0=bc[:],
                            in1=ft[b, h][:, csl],
                            scale=0.6,
                            bias=1.0,
                        )
                # pointer_feat column b: (sum q - sum feat) * u
                for h in range(NCH):
                    sq_ = tp.tile([128, 1], F32, name=f"sq{b}{h}")
                    v.tensor_reduce(sq_[:], accq[h][:], mybir.AxisListType.X, Alu.add)
                    v.tensor_sub(sq_[:], sq_[:], sf[b, h][:])
                    v.tensor_mul(pfT[h][:, b : b + 1], sq_[:], u_bc[:, b : b + 1])
                    v.tensor_copy(fus[h][:, b : b + 1], pfT[h][:, b : b + 1])
                if b == 1:
                    mlp_pair(0, 2)
                    finish_sample(0)
                    finish_sample(1)
                elif b == 3:
                    mlp_pair(2, 2)
                    finish_sample(2)
                    finish_sample(3)

            for m in range(2):
                nc.sync.dma_start(glang_d[m * 128 : (m + 1) * 128, :], glT[m][:])
                nc.sync.dma_start(pf_d[m * 128 : (m + 1) * 128, :], pfT[m][:])

    nc.finalize()
    return nc


_NC_CACHE = None


def _get_nc():
    global _NC_CACHE
    if _NC_CACHE is None:
        _NC_CACHE = _build_nc()
    return _NC_CACHE


def _prep_in_maps(feat, lang_vec, pointer_xywh, conv1_w, conv1_b, conv2_w, conv2_b,
                  cg_w1, cg_b1, cg_w2, cg_b2, lf_w1, lf_b1, lf_w2, lf_b2):
    f32 = np.float32
    feat = np.ascontiguousarray(np.asarray(feat, f32).reshape(B, C, HW))
    lang_vec = np.asarray(lang_vec, f32)
    pointer_xywh = np.asarray(pointer_xywh, f32)

    w1flat = np.asarray(conv1_w, f32).reshape(16, 9)
    w2vec = np.asarray(conv2_w, f32).reshape(16)

    cblob = np.zeros((64, 880), f32)
    for b in range(BPC):
        for j in range(9):
            cblob[j * BPC + b, b * 16 : (b + 1) * 16] = w1flat[:, j]   # c1lhsT
        cblob[b * 16 : (b + 1) * 16, 64 + b] = w2vec                   # c2lhsT
        cblob[b, 69] = np.asarray(conv2_b, f32)[0]                     # c2b
        for j in range(9):
            cblob[b, 198 + j * BPC + b] = 1.0                          # rep436
        cblob[b, 234 + b] = 1.0                                        # id4
        cblob[b, 240 + b * 128 : 240 + (b + 1) * 128] = 1.0            # sel2 hi
        cblob[32 + b, 240 + b * 128 : 240 + (b + 1) * 128] = 1.0       # sel2 lo
    cblob[:, 68] = np.tile(np.asarray(conv1_b, f32), BPC)              # c1b
    cblob[0, 70:198] = 1.0                                             # ones128
    for ky in range(3):
        for kx in range(3):
            j = ky * 3 + kx
            for b in range(BPC):
                cblob[j * BPC + b, 752:816] = XS_EXT[kx : kx + 64]     # xgs
                cblob[j * BPC + b, 816:880] = XS_EXT[ky : ky + 64]     # ygs

    wblob = np.empty((128, 3072), ml_dtypes.bfloat16)
    mats = ([np.asarray(cg_w1, f32).T[k * 128 : (k + 1) * 128] for k in range(4)]
            + [np.asarray(lf_w1, f32).T[k * 128 : (k + 1) * 128] for k in range(4)]
            + [np.asarray(cg_w2, f32).T[k * 128 : (k + 1) * 128] for k in range(2)]
            + [np.asarray(lf_w2, f32).T[k * 128 : (k + 1) * 128] for k in range(2)])
    for k, m in enumerate(mats):
        wblob[:, k * 256 : (k + 1) * 256] = m

    bblob = np.empty((128, 8), f32)
    for i, bv in enumerate([cg_b1, cg_b2, lf_b1, lf_b2]):
        bblob[:, 2 * i : 2 * i + 2] = np.asarray(bv, f32).reshape(2, 128).T

    com = {
        "cblob": cblob,
        "wblob": wblob,
        "bblob": bblob,
    }
    in_maps = []
    for i in range(N_CORES):
        sl = slice(i * BPC, (i + 1) * BPC)
        m = dict(com)
        m["feat"] = np.ascontiguousarray(feat[sl])
        m["xywh"] = np.ascontiguousarray(pointer_xywh[sl])
        lT = np.ascontiguousarray(lang_vec[sl].T)  # [256, 4]
        lblob = np.empty((128, 8), f32)
        lblob[:, 0:4] = lT[0:128]
        lblob[:, 4:8] = lT[128:256]
        m["lblob"] = lblob
        in_maps.append(m)
    return in_maps


def _gather(results):
    f32 = np.float32
    guided = np.concatenate([r["guided"] for r in results], 0).reshape(B, C, H, W)
    glang = np.concatenate([r["glangT"].T for r in results], 0)
    boxes = np.concatenate([r["boxes"] for r in results], 0)
    rmask = np.concatenate([r["rmask"] for r in results], 0).reshape(B, 1, H, W)
    pf = np.concatenate([r["pfT"].T for r in results], 0)
    return (
        guided.astype(f32, copy=False),
        glang.astype(f32, copy=False),
        boxes.astype(f32, copy=False),
        rmask.astype(f32, copy=False),
        pf.astype(f32, copy=False),
    )


def kernel(**inputs):
    global LAST_RESULTS
    nc = _get_nc()
    in_maps = _prep_in_maps(**inputs)
    res = run_bass_kernel_spmd(nc, in_maps, core_ids=list(range(N_CORES)))
    LAST_RESULTS = res
    return _gather(res.results)


# revision 29
# speedup vs baseline: 1.0968x; 1.0443x over previous
"""PointerGuidance Trainium2 kernel.

Data-parallel over batch: 32 samples -> 8 NeuronCores x 4 samples.
Each core processes its 4 samples end-to-end; outputs are concatenated on host.

Device dataflow per core (feat shard [4, 256, 4096] f32):
  1. Box math + margin (tiny DVE/ACT ops on 4 partitions), boxes_xyxy out.
  2. Rectangle masks for all 9 conv-taps x 4 samples built as [36, 4096]
     indicator tensor via compares against host-precomputed shifted grids.
  3. conv3x3 (1->16) + relu and conv1x1 (16->1) + sigmoid as PE matmuls with
     block-diagonal weights -> refined_mask [4, 4096].
  4. Per feat tile [128, 4096]: ACT accumulates sum(feat); PE broadcasts the
     sample's mask row across 128 partitions (K=1 matmul into PSUM); one fused
     DVE affine_mul_reduce computes q = (0.6*mask+1)*feat in place and
     accumulates sum(q). pointer_feat = (sum(q)-sum(feat)) / (0.6*max(area,1)).
  5. Per-sample MLPs (channel gate + lang fuse) on PE with N=1 matmuls.
  6. In-place scale q *= (1+0.5*ch_gate) (per-partition scalar; split between
     ACT and DVE), DMA out.
"""

import os
import sys

import numpy as np
import ml_dtypes

for _p in ("/opt/trn_rl_repo",):
    if os.path.isdir(_p) and _p not in sys.path:
        sys.path.insert(0, _p)

import concourse.bass as bass
import concourse.bacc as bacc
import concourse.mybir as mybir
import concourse.tile as tile
from concourse.bass_utils import run_bass_kernel_spmd

N_CORES = 8
B, C, H, W = 32, 256, 64, 64
BPC = B // N_CORES  # samples per core
HW = H * W
NCH = 2  # channel halves of 128

dt = mybir.dt
F32 = dt.float32
F32R = dt.float32r
BF16 = dt.bfloat16
Alu = mybir.AluOpType
Act = mybir.ActivationFunctionType

# jnp.linspace(0, 1, 64) == arange(64, f32) * f32(1/63) (verified bit-exact)
XS = (np.arange(64, dtype=np.float32) * np.float32(1.0 / 63.0)).astype(np.float32)
# sentinels for out-of-range taps: strictly below 0 / above 1 so the
# (>= lo) / (<= hi) compares are false for any clipped bound
XS_EXT = np.concatenate([[np.float32(-1.0)], XS, [np.float32(2.0)]]).astype(np.float32)

LAST_RESULTS = None  # BassKernelResults of the most recent run (for profiling)


def _build_grids():
    """[36, 4096] grids of shifted x / y linspace values (row j*4+b, tap j=(ky,kx))."""
    xg = np.empty((36, HW), np.float32)
    yg = np.empty((36, HW), np.float32)
    for ky in range(3):
        for kx in range(3):
            j = ky * 3 + kx
            xrow = np.tile(XS_EXT[kx : kx + 64], 64)  # value xs[x + (kx-1)]
            yrow = np.repeat(XS_EXT[ky : ky + 64], 64)  # value ys[y + (ky-1)]
            for b in range(BPC):
                xg[j * BPC + b] = xrow
                yg[j * BPC + b] = yrow
    return xg, yg


_XG36, _YG36 = _build_grids()


def _build_nc():
    nc = bacc.Bacc()

    # ---- DRAM I/O -------------------------------------------------------
    feat_d = nc.dram_tensor("feat", [BPC, C, HW], F32, kind="ExternalInput")
    xywh_d = nc.dram_tensor("xywh", [BPC, 4], F32, kind="ExternalInput")
    # cblob [64, 880]: c1lhsT | c2lhsT | c1b | c2b | ones128 | rep436 | id4 |
    #                  sel4 | xgs[36,64] | ygs[36,64]
    cblob_d = nc.dram_tensor("cblob", [64, 880], F32, kind="ExternalInput")
    # wblob [128, 3072]: wcg1T(4) | wlf1T(4) | wcg2T(2) | wlf2T(2), each [128,256]
    wblob_d = nc.dram_tensor("wblob", [128, 3072], BF16, kind="ExternalInput")
    # bblob [128, 8]: bcg1 | bcg2 | blf1 | blf2, each [128,2]
    bblob_d = nc.dram_tensor("bblob", [128, 8], F32, kind="ExternalInput")
    # lblob [128, 8]: langT rows 0:128 | rows 128:256
    lblob_d = nc.dram_tensor("lblob", [128, 8], F32, kind="ExternalInput")

    guided_d = nc.dram_tensor("guided", [BPC, C, HW], F32, kind="ExternalOutput")
    glang_d = nc.dram_tensor("glangT", [C, BPC], F32, kind="ExternalOutput")
    boxes_d = nc.dram_tensor("boxes", [BPC, 4], F32, kind="ExternalOutput")
    rmask_d = nc.dram_tensor("rmask", [BPC, HW], F32, kind="ExternalOutput")
    pf_d = nc.dram_tensor("pfT", [C, BPC], F32, kind="ExternalOutput")

    v = nc.vector
    s = nc.scalar
    pe = nc.tensor

    with tile.TileContext(nc) as tc:
        with (
            tc.tile_pool(name="const", bufs=1) as cp,
            tc.tile_pool(name="tiny", bufs=1) as tp,
            tc.tile_pool(name="featp", bufs=8) as fp,
            tc.tile_pool(name="mskp", bufs=1) as bp,
            tc.tile_pool(name="hidp", bufs=1) as hp,
            tc.tile_pool(name="rmrp", bufs=1) as rp,
            tc.tile_pool(name="psA", bufs=3, space="PSUM") as psA,
            tc.tile_pool(name="psB", bufs=2, space="PSUM") as psB,
        ):
            # ---- load constants: grids first (they gate the mask
            # pipeline), then packed blobs (few DMA triggers) -------------
            xw = cp.tile([BPC, 4], F32, name="xw")
            nc.sync.dma_start(xw[:], xywh_d[:])
            cb = cp.tile([64, 880], F32, name="cb")
            nc.sync.dma_start(cb[:], cblob_d[:])
            c1l = cb[0:36, 0:64]
            c2l = cb[:, 64:68]
            c1b = cb[:, 68:69]
            c2b = cb[0:BPC, 69:70]
            ones = cb[0:1, 70:198]
            rep = cb[0:4, 198:234]
            id4 = cb[0:4, 234:238]
            sel2 = cb[0:36, 240:752]
            xgs = cb[0:36, 752:816]
            ygs = cb[0:36, 816:880]
            wb_ = cp.tile([128, 3072], BF16, name="wb_")
            nc.sync.dma_start(wb_[:], wblob_d[:])
            wcg1 = [wb_[:, (k) * 256 : (k + 1) * 256] for k in range(4)]
            wlf1 = [wb_[:, (4 + k) * 256 : (5 + k) * 256] for k in range(4)]
            wcg2 = [wb_[:, (8 + k) * 256 : (9 + k) * 256] for k in range(2)]
            wlf2 = [wb_[:, (10 + k) * 256 : (11 + k) * 256] for k in range(2)]
            bb = cp.tile([128, 8], F32, name="bb")
            nc.sync.dma_start(bb[:], bblob_d[:])
            bcg1 = bb[:, 0:2]
            bcg2 = bb[:, 2:4]
            blf1 = bb[:, 4:6]
            blf2 = bb[:, 6:8]
            lb = cp.tile([128, 8], F32, name="lb")
            nc.sync.dma_start(lb[:], lblob_d[:])
            lT = [lb[:, 0:4], lb[:, 4:8]]
            lbb = tp.tile([128, 8], BF16, name="lbb")
            v.tensor_copy(lbb[:], lb[:])
            lTb = [lbb[:, 0:4], lbb[:, 4:8]]

            # ---- box math (rows = samples, [4,1] columns) --------------
            def tnew(name):
                return tp.tile([BPC, 1], F32, name=name)

            def clamp01(dst, src):
                v.tensor_scalar(dst[:], src[:], 0.0, 1.0, Alu.max, Alu.min)

            xc, yc, wv, hv = (xw[:, i : i + 1] for i in range(4))
            hw_ = tnew("hw_")
            hh_ = tnew("hh_")
            v.tensor_scalar(hw_[:], wv, 0.5, None, Alu.mult)
            v.tensor_scalar(hh_[:], hv, 0.5, None, Alu.mult)
            x1, x2, y1, y2 = tnew("x1"), tnew("x2"), tnew("y1"), tnew("y2")
            v.tensor_sub(x1[:], xc, hw_[:])
            v.tensor_add(x2[:], xc, hw_[:])
            v.tensor_sub(y1[:], yc, hh_[:])
            v.tensor_add(y2[:], yc, hh_[:])
            x1c, x2c, y1c, y2c = tnew("x1c"), tnew("x2c"), tnew("y1c"), tnew("y2c")
            clamp01(x1c, x1)
            clamp01(x2c, x2)
            clamp01(y1c, y1)
            clamp01(y2c, y2)
            xlo, xhi, ylo, yhi = tnew("xlo"), tnew("xhi"), tnew("ylo"), tnew("yhi")
            v.tensor_tensor(xlo[:], x1c[:], x2c[:], Alu.min)
            v.tensor_max(xhi[:], x1c[:], x2c[:])
            v.tensor_tensor(ylo[:], y1c[:], y2c[:], Alu.min)
            v.tensor_max(yhi[:], y1c[:], y2c[:])
            wb, hb = tnew("wb"), tnew("hb")
            v.tensor_sub(wb[:], xhi[:], xlo[:])
            v.tensor_scalar(wb[:], wb[:], 1e-06, None, Alu.max)
            v.tensor_sub(hb[:], yhi[:], ylo[:])
            v.tensor_scalar(hb[:], hb[:], 1e-06, None, Alu.max)
            cx, cy = tnew("cx"), tnew("cy")
            v.tensor_add(cx[:], xhi[:], xlo[:])
            v.tensor_scalar(cx[:], cx[:], 0.5, None, Alu.mult)
            v.tensor_add(cy[:], yhi[:], ylo[:])
            v.tensor_scalar(cy[:], cy[:], 0.5, None, Alu.mult)
            wbh, hbh = tnew("wbh"), tnew("hbh")
            v.tensor_scalar(wbh[:], wb[:], 0.5, None, Alu.mult)
            v.tensor_scalar(hbh[:], hb[:], 0.5, None, Alu.mult)
            boxes_sb = tp.tile([BPC, 4], F32, name="boxes_sb")
            bx1, by1, bx2, by2 = (boxes_sb[:, i : i + 1] for i in range(4))
            tmp = tnew("tmpbox")
            v.tensor_sub(tmp[:], cx[:], wbh[:])
            clamp01(bx1, tmp)
            v.tensor_sub(tmp[:], cy[:], hbh[:])
            clamp01(by1, tmp)
            v.tensor_add(tmp[:], cx[:], wbh[:])
            clamp01(bx2, tmp)
            v.tensor_add(tmp[:], cy[:], hbh[:])
            clamp01(by2, tmp)
            nc.sync.dma_start(boxes_d[:], boxes_sb[:])

            # margin = clip(0.2*sqrt(w^2+h^2), 0.02, 0.2), w/h from clamped box
            wm, hm = tnew("wm"), tnew("hm")
            v.tensor_sub(wm[:], bx2, bx1)
            v.tensor_scalar(wm[:], wm[:], 1e-4, None, Alu.max)
            v.tensor_sub(hm[:], by2, by1)
            v.tensor_scalar(hm[:], hm[:], 1e-4, None, Alu.max)
            d2 = tnew("d2")
            v.tensor_mul(wm[:], wm[:], wm[:])
            v.tensor_mul(hm[:], hm[:], hm[:])
            v.tensor_add(d2[:], wm[:], hm[:])
            sq = tnew("sq")
            s.sqrt(sq[:], d2[:])
            # two Newton iterations: s <- 0.5*(s + d2/s), to match IEEE sqrt
            rcp = tnew("rcp")
            qn = tnew("qn")
            for _ in range(2):
                v.reciprocal(rcp[:], sq[:])
                v.tensor_mul(qn[:], d2[:], rcp[:])
                v.tensor_add(sq[:], sq[:], qn[:])
                v.tensor_scalar(sq[:], sq[:], 0.5, None, Alu.mult)
            margin = tnew("margin")
            v.tensor_scalar(margin[:], sq[:], 0.2, None, Alu.mult)
            v.tensor_scalar(margin[:], margin[:], 0.02, 0.2, Alu.max, Alu.min)
            bnd4 = tp.tile([BPC, 4], F32, name="bnd4")
            v.tensor_sub(tmp[:], bx1, margin[:])
            clamp01(bnd4[:, 0:1], tmp)
            v.tensor_add(tmp[:], bx2, margin[:])
            clamp01(bnd4[:, 1:2], tmp)
            v.tensor_sub(tmp[:], by1, margin[:])
            clamp01(bnd4[:, 2:3], tmp)
            v.tensor_add(tmp[:], by2, margin[:])
            clamp01(bnd4[:, 3:4], tmp)

            # replicate bounds to 36 partitions: rep.T @ bnd4
            bnd_ps = psB.tile([36, 4], F32, name="bnd_ps", tag="ps1")
            pe.matmul(bnd_ps[:], rep, bnd4[:], start=True, stop=True)
            bnd36 = tp.tile([36, 4], F32, name="bnd36")
            v.tensor_copy(bnd36[:], bnd_ps[:])

            # rounded copies of conv lhsTs and sel (tiny one-time ops)
            c1lr = tp.tile([36, 64], F32R, name="c1lr")
            v.tensor_copy(c1lr[:], c1l)
            c2lr = tp.tile([64, BPC], F32R, name="c2lr")
            v.tensor_copy(c2lr[:], c2l)
            sel2r = tp.tile([36, 4 * 128], BF16, name="sel2r")
            v.tensor_copy(sel2r[:], sel2)

            # ---- shifted rectangle masks: compare on [36,64] mini-grids,
            # expand to [36,4096] with stride-0 APs in one multiply -------
            xgate = tp.tile([36, 64], F32, name="xgate")
            ygate = tp.tile([36, 64], F32, name="ygate")
            v.tensor_scalar(xgate[:], xgs, bnd36[:, 0:1], None, Alu.is_ge)
            v.scalar_tensor_tensor(xgate[:], xgs, bnd36[:, 1:2], xgate[:], Alu.is_le, Alu.mult)
            v.tensor_scalar(ygate[:], ygs, bnd36[:, 2:3], None, Alu.is_ge)
            v.scalar_tensor_tensor(ygate[:], ygs, bnd36[:, 3:4], ygate[:], Alu.is_le, Alu.mult)
            msk = bp.tile([36, HW], F32R, name="msk", tag="mskt")
            xg_exp = bass.AP(xgate.tensor, xgate[:].offset, [xgate[:].ap[0], [0, 64], [1, 64]])
            yg_exp = bass.AP(ygate.tensor, ygate[:].offset, [ygate[:].ap[0], [1, 64], [0, 64]])
            v.tensor_tensor(msk[:].rearrange("p (a b) -> p a b", a=64), xg_exp, yg_exp, Alu.mult)

            # ---- conv1 3x3 (K=36 matmul) + relu ------------------------
            hidden = hp.tile([64, HW], F32R, name="hidden", tag="hid")
            for k in range(8):
                sl = slice(k * 512, (k + 1) * 512)
                ps = psB.tile([64, 512], F32, name="convps", tag="ps1")
                pe.matmul(ps[:], c1lr[:], msk[:, sl], start=True, stop=True)
                v.tensor_scalar(hidden[:, sl], ps[:], c1b, 0.0, Alu.add, Alu.max)

            # ---- conv2 1x1 (K=64 matmul) + sigmoid ---------------------
            rm4 = bp.tile([BPC, HW], F32, name="rm4", tag="mskt")
            for k in range(8):
                sl = slice(k * 512, (k + 1) * 512)
                ps2 = psB.tile([BPC, 512], F32, name="conv2ps", tag="ps1")
                pe.matmul(ps2[:], c2lr[:], hidden[:, sl], start=True, stop=True)
                s.activation(rm4[:, sl], ps2[:], Act.Sigmoid, bias=c2b)
            nc.sync.dma_start(rmask_d[:], rm4[:])
            # bf16 hi/lo split of the mask: hi+lo accumulated in PSUM by the
            # K=8 broadcast matmul recovers ~f32 precision at bf16 speed
            rmhl = rp.tile([36, HW], BF16, name="rmhl")
            nc.gpsimd.memset(rmhl[:], 0.0)
            area = tp.tile([BPC, 1], F32, name="area")
            v.tensor_scalar(rmhl[0:BPC, :], rm4[:], 0.0, 0.0, Alu.add, Alu.add, accum_out=area[:])
            v.tensor_sub(rmhl[32 : 32 + BPC, :], rm4[:], rmhl[0:BPC, :])

            # ---- mask area -> u = 1/(0.6*max(area,1)) broadcast --------
            v.tensor_scalar(area[:], area[:], 1.0, None, Alu.max)
            v.tensor_scalar(area[:], area[:], 0.6, None, Alu.mult)
            u4 = tnew("u4")
            v.reciprocal(u4[:], area[:])
            u_ps = psB.tile([1, 4], F32, name="u_ps", tag="ps1")
            pe.matmul(u_ps[:], u4[:], id4, start=True, stop=True)
            u_row = tp.tile([1, 4], F32, name="u_row")
            v.tensor_copy(u_row[:], u_ps[:])
            ubc_ps = psB.tile([128, 4], F32, name="ubc_ps", tag="ps1")
            pe.matmul(ubc_ps[:], ones, u_row[:], start=True, stop=True)
            u_bc = tp.tile([128, 4], F32, name="u_bc")
            v.tensor_copy(u_bc[:], ubc_ps[:])

            # ---- feat in + sum(feat) via in-place identity copy --------
            ft = {}
            sf = {}
            for b in range(BPC):
                for h in range(NCH):
                    t = fp.tile([128, HW], F32, name="ftile")
                    nc.sync.dma_start(t[:], feat_d[b, h * 128 : (h + 1) * 128, :])
                    ft[b, h] = t
                    acc = tp.tile([128, 1], F32, name=f"sf{b}{h}")
                    s.activation(t[:], t[:], Act.Copy, accum_out=acc[:])
                    sf[b, h] = acc

            # ---- main loop: broadcast mask, fused q & sum(q), MLPs -----
            fus = [tp.tile([128, BPC], BF16, name=f"fus{h}") for h in range(2)]
            pfT = [tp.tile([128, BPC], F32, name=f"pfT{h}") for h in range(2)]
            s1t = [tp.tile([128, BPC], F32, name=f"s1t{h}") for h in range(2)]
            glT = [tp.tile([128, BPC], F32, name=f"glT{m}") for m in range(2)]

            def mlp_pair(b0, n):
                """Batched MLPs for samples [b0, b0+n) (rhs columns b0:b0+n)."""
                rhs_ch = [fus[0], fus[1], lTb[0], lTb[1]]
                csl = slice(b0, b0 + n)

                def mlp(w1, w2, b1, b2, tag):
                    h1sb = []
                    for m in range(2):
                        h1ps = psB.tile([128, n], F32, name=f"h1ps{tag}{b0}{m}", tag="ps1")
                        for kc in range(4):
                            pe.matmul(
                                h1ps[:],
                                w1[kc][:, m * 128 : (m + 1) * 128],
                                rhs_ch[kc][:, csl],
                                start=(kc == 0),
                                stop=(kc == 3),
                            )
                        hh = tp.tile([128, n], BF16, name=f"h1sb{tag}{b0}{m}")
                        s.activation(hh[:], h1ps[:], Act.Relu, bias=b1[:, m : m + 1])
                        h1sb.append(hh)
                    out2 = []
                    for m in range(2):
                        h2ps = psB.tile([128, n], F32, name=f"h2ps{tag}{b0}{m}", tag="ps1")
                        for kc in range(2):
                            pe.matmul(
                                h2ps[:],
                                w2[kc][:, m * 128 : (m + 1) * 128],
                                h1sb[kc][:],
                                start=(kc == 0),
                                stop=(kc == 1),
                            )
                        out2.append(h2ps)
                    return out2

                cg2 = mlp(wcg1, wcg2, bcg1, bcg2, "cg")
                for m in range(2):
                    g = tp.tile([128, n], F32, name=f"g{b0}{m}")
                    s.activation(g[:], cg2[m][:], Act.Sigmoid, bias=bcg2[:, m : m + 1])
                    v.tensor_scalar(s1t[m][:, csl], g[:], 0.5, 1.0, Alu.mult, Alu.add)
                lf2 = mlp(wlf1, wlf2, blf1, blf2, "lf")
                for m in range(2):
                    tn = tp.tile([128, n], F32, name=f"tn{b0}{m}")
                    s.activation(tn[:], lf2[m][:], Act.Tanh, bias=blf2[:, m : m + 1])
                    v.scalar_tensor_tensor(
                        glT[m][:, csl], tn[:], 0.4, lT[m][:, csl], Alu.mult, Alu.add
                    )

            def finish_sample(b):
                """Channel-gate scale (in place) + store guided tiles of b."""
                for h in range(NCH):
                    if b < 2:
                        s.activation(
                            ft[b, h][:], ft[b, h][:], Act.Copy, scale=s1t[h][:, b : b + 1]
                        )
                    else:
                        v.tensor_scalar(
                            ft[b, h][:], ft[b, h][:], s1t[h][:, b : b + 1], None, Alu.mult
                        )
                    nc.sync.dma_start(
                        guided_d[b, h * 128 : (h + 1) * 128, :], ft[b, h][:]
                    )

            for b in range(BPC):
                accq = {h: tp.tile([128, 4], F32, name=f"accq{b}{h}") for h in range(NCH)}
                for ck in range(4):
                    csl = slice(ck * 1024, (ck + 1) * 1024)
                    bc = psA.tile([128, 1024], F32, name="bc", tag="bc")
                    for s2 in range(2):
                        lo = ck * 1024 + s2 * 512
                        pe.matmul(
                            bc[:, s2 * 512 : (s2 + 1) * 512],
                            sel2r[:, b * 128 : (b + 1) * 128],
                            rmhl[:, lo : lo + 512],
                            start=True,
                            stop=True,
                        )
                    for h in range(NCH):
                        v.affine_mul_reduce(
                            out=ft[b, h][:, csl],
                            accum_out=accq[h][:, ck : ck + 1],
                            in0=bc[:],
                            in1=ft[b, h][:, csl],
                            scale=0.6,
                            bias=1.0,
                        )
                # pointer_feat column b: (sum q - sum feat) * u
                for h in range(NCH):
                    sq_ = tp.tile([128, 1], F32, name=f"sq{b}{h}")
                    v.tensor_reduce(sq_[:], accq[h][:], mybir.AxisListType.X, Alu.add)
                    v.tensor_sub(sq_[:], sq_[:], sf[b, h][:])
                    v.tensor_mul(pfT[h][:, b : b + 1], sq_[:], u_bc[:, b : b + 1])
                    v.tensor_copy(fus[h][:, b : b + 1], pfT[h][:, b : b + 1])
                mlp_pair(b, 1)
                finish_sample(b)

            for m in range(2):
                nc.sync.dma_start(glang_d[m * 128 : (m + 1) * 128, :], glT[m][:])
                nc.sync.dma_start(pf_d[m * 128 : (m + 1) * 128, :], pfT[m][:])

    nc.finalize()
    return nc


_NC_CACHE = None


def _get_nc():
    global _NC_CACHE
    if _NC_CACHE is None:
        _NC_CACHE = _build_nc()
    return _NC_CACHE


def _prep_in_maps(feat, lang_vec, pointer_xywh, conv1_w, conv1_b, conv2_w, conv2_b,
                  cg_w1, cg_b1, cg_w2, cg_b2, lf_w1, lf_b1, lf_w2, lf_b2):
    f32 = np.float32
    feat = np.ascontiguousarray(np.asarray(feat, f32).reshape(B, C, HW))
    lang_vec = np.asarray(lang_vec, f32)
    pointer_xywh = np.asarray(pointer_xywh, f32)

    w1flat = np.asarray(conv1_w, f32).reshape(16, 9)
    w2vec = np.asarray(conv2_w, f32).reshape(16)

    cblob = np.zeros((64, 880), f32)
    for b in range(BPC):
        for j in range(9):
            cblob[j * BPC + b, b * 16 : (b + 1) * 16] = w1flat[:, j]   # c1lhsT
        cblob[b * 16 : (b + 1) * 16, 64 + b] = w2vec                   # c2lhsT
        cblob[b, 69] = np.asarray(conv2_b, f32)[0]                     # c2b
        for j in range(9):
            cblob[b, 198 + j * BPC + b] = 1.0                          # rep436
        cblob[b, 234 + b] = 1.0                                        # id4
        cblob[b, 240 + b * 128 : 240 + (b + 1) * 128] = 1.0            # sel2 hi
        cblob[32 + b, 240 + b * 128 : 240 + (b + 1) * 128] = 1.0       # sel2 lo
    cblob[:, 68] = np.tile(np.asarray(conv1_b, f32), BPC)              # c1b
    cblob[0, 70:198] = 1.0                                             # ones128
    for ky in range(3):
        for kx in range(3):
            j = ky * 3 + kx
            for b in range(BPC):
                cblob[j * BPC + b, 752:816] = XS_EXT[kx : kx + 64]     # xgs
                cblob[j * BPC + b, 816:880] = XS_EXT[ky : ky + 64]     # ygs

    wblob = np.empty((128, 3072), ml_dtypes.bfloat16)
    mats = ([np.asarray(cg_w1, f32).T[k * 128 : (k + 1) * 128] for k in range(4)]
            + [np.asarray(lf_w1, f32).T[k * 128 : (k + 1) * 128] for k in range(4)]
            + [np.asarray(cg_w2, f32).T[k * 128 : (k + 1) * 128] for k in range(2)]
            + [np.asarray(lf_w2, f32).T[k * 128 : (k + 1) * 128] for k in range(2)])
    for k, m in enumerate(mats):
        wblob[:, k * 256 : (k + 1) * 256] = m

    bblob = np.empty((128, 8), f32)
    for i, bv in enumerate([cg_b1, cg_b2, lf_b1, lf_b2]):
        bblob[:, 2 * i : 2 * i + 2] = np.asarray(bv, f32).reshape(2, 128).T

    com = {
        "cblob": cblob,
        "wblob": wblob,
        "bblob": bblob,
    }
    in_maps = []
    for i in range(N_CORES):
        sl = slice(i * BPC, (i + 1) * BPC)
        m = dict(com)
        m["feat"] = np.ascontiguousarray(feat[sl])
        m["xywh"] = np.ascontiguousarray(pointer_xywh[sl])
        lT = np.ascontiguousarray(lang_vec[sl].T)  # [256, 4]
        lblob = np.empty((128, 8), f32)
        lblob[:, 0:4] = lT[0:128]
        lblob[:, 4:8] = lT[128:256]
        m["lblob"] = lblob
        in_maps.append(m)
    return in_maps


def _gather(results):
    f32 = np.float32
    guided = np.concatenate([r["guided"] for r in results], 0).reshape(B, C, H, W)
    glang = np.concatenate([r["glangT"].T for r in results], 0)
    boxes = np.concatenate([r["boxes"] for r in results], 0)
    rmask = np.concatenate([r["rmask"] for r in results], 0).reshape(B, 1, H, W)
    pf = np.concatenate([r["pfT"].T for r in results], 0)
    return (
        guided.astype(f32, copy=False),
        glang.astype(f32, copy=False),
        boxes.astype(f32, copy=False),
        rmask.astype(f32, copy=False),
        pf.astype(f32, copy=False),
    )


def kernel(**inputs):
    global LAST_RESULTS
    nc = _get_nc()
    in_maps = _prep_in_maps(**inputs)
    res = run_bass_kernel_spmd(nc, in_maps, core_ids=list(range(N_CORES)))
    LAST_RESULTS = res
    return _gather(res.results)
